# revision 1
# baseline (speedup 1.0000x reference)
"""Bipartite 2-layer SAGEConv GNN on 8 Trainium2 NeuronCores.

Strategy:
  - Edges sharded by destination range (core c owns dst rows [S*c, S*(c+1))
    for BOTH directions, so layer-2 lin_r terms stay core-local).
  - Per core+direction, dsts are sorted by degree; edges packed into 8-slot
    segments, 16 dst-rows per PSUM block, variable tiles per block
    (schedule = max over cores, so one SPMD program serves all cores).
  - Message gather: dma_gather with a CENTERED table base (idx int16 signed,
    idx = node - N/2) so all 50001 rows are addressable.
  - Segment-sum: PE matmul with constant one-hot lhsT R8 [128 slots, 16 rows]
    accumulated in PSUM per block (no scatter-add races).
  - Layer 2 transform-first: z = x1 @ w2l.T (64 wide) gathered instead of x1.
  - One AllGather per z table; everything else core-local.
  - Degree-permutation undone at DRAM stores via unique-index dma_scatter_add.
"""
import sys
import numpy as np

sys.path.insert(0, "/opt/trn_rl_repo")

# ---------------- problem dims (hardcoded for the harness) ----------------
N = 50000
E = 800000
F_IN = 128
HID = 256
CLS = 64
NCORES = 8

SEG = 4            # slots per segment (one dst's edges per tile-row)
BPD = 32           # dsts per psum block (32-partition alignment for engine ops)
CHUNK_TILES = 8    # tiles per gather call (1024 idx = HW SWDGE ring limit)
SCAT_CHUNK = 512   # rows per scatter-add call (2 read descs per row)


class CFG:
    def __init__(self, n=N, e=E, center=None):
        self.N = n
        self.E = e
        self.S = n // NCORES          # dst rows per core
        self.CENTER = n // 2 if center is None else center  # gather table base row
        self.ZROW = n                 # zero row index (centered: n - CENTER >= 0)
        self.NB = -(-self.S // BPD)   # blocks per direction
        self.RT = -(-self.S // 128)   # 128-row tiles of the slice
        self.SP = self.RT * 128       # padded rows


# ---------------- host-side edge scheduling ----------------

def _prep_dir(src_g, dst_g, c, cfg):
    """Per-core, per-direction metadata."""
    lo = c * cfg.S
    m = (dst_g >= lo) & (dst_g < lo + cfg.S)
    ls = src_g[m].astype(np.int64)
    ld = (dst_g[m] - lo).astype(np.int64)
    deg = np.bincount(ld, minlength=cfg.S)
    pi = np.argsort(-deg, kind="stable").astype(np.int64)
    order = np.argsort(ld, kind="stable")
    ls_s = ls[order]
    starts = np.zeros(cfg.S + 1, np.int64)
    starts[1:] = np.cumsum(deg)
    degp = np.zeros(cfg.NB * BPD, np.int64)
    degp[: cfg.S] = deg[pi]
    treq = np.maximum(
        1, -(-degp.reshape(cfg.NB, BPD).max(1) // SEG)
    ).astype(np.int64)
    return dict(pi=pi, deg=deg, starts=starts, ls_s=ls_s, degp=degp, treq=treq)


def _n_tiles(T):
    return int(T.sum())


def _build_slots(meta, T, cfg):
    """Slot array (src node ids, ZROW for dummies) per the shared schedule.

    Also guarantees every CHUNK_TILES-aligned tile boundary ends with a slot
    whose centered index is non-negative (the SWDGE ucode trims trailing
    negatives), swapping within a dst row -- or dst rows within the block --
    when needed. meta["pi"] is mutated accordingly.
    """
    pi, deg, starts, ls_s = meta["pi"], meta["deg"], meta["starts"], meta["ls_s"]
    total_tiles = int(T.sum())
    out = np.full((total_tiles, BPD, SEG), cfg.ZROW, np.int64)
    row_of_tile = np.zeros(total_tiles, np.int64)   # block index per tile
    t0 = 0
    blk_start = {}
    for b in range(cfg.NB):
        tb = int(T[b])
        blk_start[b] = t0
        row_of_tile[t0:t0 + tb] = b
        blk = out[t0 : t0 + tb]          # [tb, BPD, SEG]
        for mrow in range(BPD):
            r = BPD * b + mrow
            if r >= cfg.S:
                continue
            D = int(pi[r])
            d = int(deg[D])
            if d == 0:
                continue
            vals = np.full(tb * SEG, cfg.ZROW, np.int64)
            vals[:d] = ls_s[starts[D] : starts[D] + d]
            blk[:, mrow, :] = vals.reshape(tb, SEG)
        t0 += tb
    # fix chunk tails: final slot of tiles CHUNK_TILES-1, 2*CHUNK_TILES-1, ...
    def row_get(blk, m, j):
        return blk[j // SEG, m, j % SEG]

    def row_swap(blk, m, j1, j2):
        a, b_ = blk[j1 // SEG, m, j1 % SEG], blk[j2 // SEG, m, j2 % SEG]
        blk[j1 // SEG, m, j1 % SEG] = b_
        blk[j2 // SEG, m, j2 % SEG] = a

    for tg in range(CHUNK_TILES - 1, total_tiles, CHUNK_TILES):
        b = int(row_of_tile[tg])
        tb = int(T[b])
        blk = out[blk_start[b] : blk_start[b] + tb]
        tl = tg - blk_start[b]
        jlast = tl * SEG + SEG - 1       # flat slot index within a row
        if blk[tl, BPD - 1, SEG - 1] >= cfg.CENTER:
            continue
        mgood = -1
        for m in range(BPD - 1, -1, -1):
            if (blk[:, m, :] >= cfg.CENTER).any():
                mgood = m
                break
        assert mgood >= 0, "no non-negative slot available for chunk tail"
        if mgood != BPD - 1:
            r1, r2 = BPD * b + mgood, BPD * b + BPD - 1
            pi[r1], pi[r2] = pi[r2], pi[r1]
            tmpv = blk[:, mgood, :].copy()
            blk[:, mgood, :] = blk[:, BPD - 1, :]
            blk[:, BPD - 1, :] = tmpv
        flat = blk[:, BPD - 1, :].reshape(-1).copy()  # contiguous copy
        j = int(np.nonzero(flat >= cfg.CENTER)[0][0])
        flat[j], flat[jlast] = flat[jlast], flat[j]
        blk[:, BPD - 1, :] = flat.reshape(tb, SEG)
    return out.reshape(total_tiles, 128)


def _wrap16(idx16):
    """[n] int16 -> [128, n/16]: idx i at partition i%16, col i//16, x8 replicas."""
    n = len(idx16)
    assert n % 16 == 0
    return np.tile(idx16.reshape(n // 16, 16).T, (8, 1)).astype(np.int16)


def _pad_idx(idx, ntot):
    out = np.full(ntot, -1, np.int64)
    out[: len(idx)] = idx
    return out


def _prep_all(inputs, cfg):
    """Host prep: per-core in_maps + the shared schedule."""
    x_user = np.asarray(inputs["x_user"], np.float32)
    x_product = np.asarray(inputs["x_product"], np.float32)
    ei = np.asarray(inputs["edge_index"]).astype(np.int64)
    u, p = ei[0], ei[1]

    metaA = [_prep_dir(u, p, c, cfg) for c in range(NCORES)]  # dst = p, src = u
    metaB = [_prep_dir(p, u, c, cfg) for c in range(NCORES)]  # dst = u, src = p

    TA = np.max([m["treq"] for m in metaA], axis=0)
    TB = np.max([m["treq"] for m in metaB], axis=0)

    def tab(x):
        t = np.zeros((cfg.N + 1, F_IN), np.float32)
        t[: cfg.N] = x
        return t

    xu_tab, xp_tab = tab(x_user), tab(x_product)

    w = {k: np.asarray(v, np.float32) for k, v in inputs.items()
         if k.startswith(("w_", "b_"))}
    shared = {
        "xu_tab": xu_tab, "xp_tab": xp_tab,
        "wu1lT": np.ascontiguousarray(w["w_u1_l"].T),
        "wu1rT": np.ascontiguousarray(w["w_u1_r"].T),
        "wp1lT": np.ascontiguousarray(w["w_p1_l"].T),
        "wp1rT": np.ascontiguousarray(w["w_p1_r"].T),
        "wu2lT": np.ascontiguousarray(w["w_u2_l"].T),
        "wu2rT": np.ascontiguousarray(w["w_u2_r"].T),
        "wp2lT": np.ascontiguousarray(w["w_p2_l"].T),
        "wp2rT": np.ascontiguousarray(w["w_p2_r"].T),
        "bu1": np.ascontiguousarray(w["b_u1"].reshape(2, 128).T),
        "bp1": np.ascontiguousarray(w["b_p1"].reshape(2, 128).T),
        "bu2": np.ascontiguousarray(w["b_u2"].reshape(CLS, 1)),
        "bp2": np.ascontiguousarray(w["b_p2"].reshape(CLS, 1)),
        "ident": np.eye(128, dtype=np.float32),
        "r8": np.repeat(np.eye(BPD, dtype=np.float32), SEG, axis=0),
    }

    in_maps = []
    for c in range(NCORES):
        d = dict(shared)
        for tag, meta, xsrc in (("A", metaA[c], x_product), ("B", metaB[c], x_user)):
            T = TA if tag == "A" else TB
            slots = _build_slots(meta, T, cfg)    # may mutate meta["pi"]
            # pad the slot array to a whole number of chunks
            nt = slots.shape[0]
            ntp = -(-nt // CHUNK_TILES) * CHUNK_TILES
            slp = np.full((ntp, 128), cfg.ZROW, np.int64)
            slp[:nt] = slots
            d[f"gidx{tag}"] = _wrap16(
                (slp.reshape(-1) - cfg.CENTER).astype(np.int16))
            pi = meta["pi"]
            d[f"unperm{tag}"] = _wrap16(
                _pad_idx(pi, cfg.SP).astype(np.int16))
            invc = np.zeros(cfg.SP, np.float32)
            invc[: cfg.S] = 1.0 / np.maximum(meta["deg"][pi], 1.0)
            d[f"invc{tag}"] = np.ascontiguousarray(
                invc.reshape(cfg.RT, 128).T)
            rows = c * cfg.S + pi
            xd = xsrc[rows]                       # [S, F] permuted dst-rows
            xdT = np.zeros((F_IN, cfg.SP), np.float32)
            xdT[:, : cfg.S] = xd.T
            d[f"xdT{tag}"] = xdT
        in_maps.append(d)

    return in_maps, TA, TB, metaA, metaB


# ---------------- device program ----------------

def _build_nc(cfg, TA, TB, local_mode=False):
    import concourse.bacc as bacc
    import concourse.mybir as mybir
    from concourse.tile import TileContext

    f32, i16 = mybir.dt.float32, mybir.dt.int16
    AF = mybir.ActivationFunctionType
    ALU = mybir.AluOpType

    nc = bacc.Bacc(None, target_bir_lowering=False, num_devices=NCORES,
                   dynamic_dma_scratch_size=49152, num_swdge_queues=1)

    S, SP, RT, NB, CENTER = cfg.S, cfg.SP, cfg.RT, cfg.NB, cfg.CENTER

    ntA = _n_tiles(TA)
    ntB = _n_tiles(TB)

    def colsA():
        return -(-ntA // CHUNK_TILES) * CHUNK_TILES * 8
    def colsB():
        return -(-ntB // CHUNK_TILES) * CHUNK_TILES * 8

    # ---- DRAM declarations ----
    t_xu = nc.dram_tensor("xu_tab", [cfg.N + 1, F_IN], f32, kind="ExternalInput")
    t_xp = nc.dram_tensor("xp_tab", [cfg.N + 1, F_IN], f32, kind="ExternalInput")
    tw = {}
    for k in ["wu1lT", "wu1rT", "wp1lT", "wp1rT"]:
        tw[k] = nc.dram_tensor(k, [F_IN, HID], f32, kind="ExternalInput")
    for k in ["wu2lT", "wu2rT", "wp2lT", "wp2rT"]:
        tw[k] = nc.dram_tensor(k, [HID, CLS], f32, kind="ExternalInput")
    for k in ["bu1", "bp1"]:
        tw[k] = nc.dram_tensor(k, [128, 2], f32, kind="ExternalInput")
    for k in ["bu2", "bp2"]:
        tw[k] = nc.dram_tensor(k, [CLS, 1], f32, kind="ExternalInput")
    t_ident = nc.dram_tensor("ident", [128, 128], f32, kind="ExternalInput")
    t_r8 = nc.dram_tensor("r8", [128, BPD], f32, kind="ExternalInput")
    t_gidxA = nc.dram_tensor("gidxA", [128, colsA()], i16, kind="ExternalInput")
    t_gidxB = nc.dram_tensor("gidxB", [128, colsB()], i16, kind="ExternalInput")
    t_unpA = nc.dram_tensor("unpermA", [128, SP // 16], i16, kind="ExternalInput")
    t_unpB = nc.dram_tensor("unpermB", [128, SP // 16], i16, kind="ExternalInput")
    t_invcA = nc.dram_tensor("invcA", [128, RT], f32, kind="ExternalInput")
    t_invcB = nc.dram_tensor("invcB", [128, RT], f32, kind="ExternalInput")
    t_xdTA = nc.dram_tensor("xdTA", [F_IN, SP], f32, kind="ExternalInput")
    t_xdTB = nc.dram_tensor("xdTB", [F_IN, SP], f32, kind="ExternalInput")

    t_xu2 = nc.dram_tensor("xu2", [SP, CLS], f32, kind="ExternalOutput")
    t_xp2 = nc.dram_tensor("xp2", [SP, CLS], f32, kind="ExternalOutput")

    st_zu = nc.dram_tensor("zu_stage", [SP, CLS], f32)
    st_zp = nc.dram_tensor("zp_stage", [SP, CLS], f32)
    st_r2A = nc.dram_tensor("r2A_stage", [SP, CLS], f32)
    st_r2B = nc.dram_tensor("r2B_stage", [SP, CLS], f32)
    aspace = "Local" if local_mode else "Shared"
    t_zuf = nc.dram_tensor("zu_full", [cfg.N + 1, CLS], f32, addr_space=aspace)
    t_zpf = nc.dram_tensor("zp_full", [cfg.N + 1, CLS], f32, addr_space=aspace)

    with TileContext(nc) as tc:
        # ---- persistent SBUF ----
        with tc.tile_pool(name="persist", bufs=1) as pp:
            sb_ident = pp.tile([128, 128], f32)
            sb_r8 = pp.tile([128, BPD], f32)
            sb_gidxA = pp.tile([128, colsA()], i16)
            sb_gidxB = pp.tile([128, colsB()], i16)
            sb_w = {}
            for k in ["wu1lT", "wu1rT", "wp1lT", "wp1rT"]:
                sb_w[k] = pp.tile([F_IN, HID], f32, tag=k, name=k)
            for k in ["wu2lT", "wu2rT", "wp2lT", "wp2rT"]:
                sb_w[k] = pp.tile([128, 2, CLS], f32, tag=k, name=k)
            for k in ["bu1", "bp1"]:
                sb_w[k] = pp.tile([128, 2], f32, tag=k, name=k)
            b2 = {}
            for k in ["bu2", "bp2"]:
                b2[k] = pp.tile([128, 1], f32, tag=k, name=k)
            sb_invcA = pp.tile([128, RT], f32)
            sb_invcB = pp.tile([128, RT], f32)
            sb_unpA = pp.tile([128, SP // 16], i16)
            sb_unpB = pp.tile([128, SP // 16], i16)

            nc.sync.dma_start(out=sb_ident[:], in_=t_ident[:])
            nc.sync.dma_start(out=sb_r8[:], in_=t_r8[:])
            nc.sync.dma_start(out=sb_gidxA[:], in_=t_gidxA[:])
            nc.sync.dma_start(out=sb_gidxB[:], in_=t_gidxB[:])
            for k, t in tw.items():
                if k in ("bu2", "bp2"):
                    nc.sync.dma_start(out=b2[k][64:64 + CLS, :], in_=t[:])
                elif k in ("wu2lT", "wu2rT", "wp2lT", "wp2rT"):
                    nc.sync.dma_start(
                        out=sb_w[k][:],
                        in_=t.rearrange("(k p) c -> p k c", p=128)[:])
                else:
                    nc.sync.dma_start(out=sb_w[k][:], in_=t[:])
            nc.sync.dma_start(out=sb_invcA[:], in_=t_invcA[:])
            nc.sync.dma_start(out=sb_invcB[:], in_=t_invcB[:])
            nc.sync.dma_start(out=sb_unpA[:], in_=t_unpA[:])
            nc.sync.dma_start(out=sb_unpB[:], in_=t_unpB[:])

            # zero the scatter-target stages (+ z_full zero row)
            with tc.tile_pool(name="zpool", bufs=1) as zp:
                zt = zp.tile([128, RT, CLS], f32)
                nc.vector.memset(zt[:], 0.0)
                for st in (st_zu, st_zp, st_r2A, st_r2B, t_xu2, t_xp2):
                    nc.sync.dma_start(
                        out=st.rearrange("(c p) f -> p c f", p=128)[:], in_=zt[:])
                nc.sync.dma_start(out=t_zuf[cfg.N:cfg.N + 1, :], in_=zt[0:1, 0, :])
                nc.sync.dma_start(out=t_zpf[cfg.N:cfg.N + 1, :], in_=zt[0:1, 0, :])

            # ================= aggregation pass emitter =================
            def agg_pass(gidx_sb, T, table_ap, elem, agg_sb, label):
                ntiles = _n_tiles(T)
                with tc.tile_pool(name=f"msg{label}", bufs=4) as mp, \
                     tc.tile_pool(name=f"aggps{label}", bufs=8, space="PSUM") as ap:
                    msgs = {}

                    def chunk_of(tg):
                        ch = tg // CHUNK_TILES
                        if ch not in msgs:
                            t0c = ch * CHUNK_TILES
                            ct = min(CHUNK_TILES, ntiles - t0c)
                            m = mp.tile([128, CHUNK_TILES, elem], f32,
                                        tag="msg", name=f"msg{label}_{ch}")
                            nc.gpsimd.dma_gather(
                                m[:, :ct, :], table_ap,
                                gidx_sb[:, 8 * t0c:8 * t0c + 8 * ct],
                                ct * 128, ct * 128, elem)
                            msgs[ch] = m
                        return msgs[ch]

                    tg = 0
                    pb = 128 // BPD
                    for b in range(cfg.NB):
                        ps = ap.tile([BPD, elem], f32, tag="ps",
                                     name=f"ps{label}_{b}")
                        for k in range(int(T[b])):
                            m = chunk_of(tg)
                            nc.tensor.matmul(
                                ps[:], sb_r8[:], m[:, tg % CHUNK_TILES, :],
                                start=(k == 0), stop=(k == int(T[b]) - 1))
                            tg += 1
                        nc.vector.tensor_copy(
                            agg_sb[BPD * (b % pb):BPD * (b % pb) + BPD,
                                   b // pb, :], ps[:])

            # ================= phase-3 emitter (per direction) =================
            # consumes agg (row-major, permuted), xdT; produces z + r2_other
            def phase3(agg_sb, xdT_t, invc_sb, wl, wr, b1k, w2l, w2r_o, b2_o,
                       st_z, st_r2o, unp_sb, label):
                with tc.tile_pool(name=f"p3{label}", bufs=1) as p3, \
                     tc.tile_pool(name=f"p3w{label}", bufs=2) as p3w, \
                     tc.tile_pool(name=f"psT{label}", bufs=2, space="PSUM") as psT, \
                     tc.tile_pool(name=f"psG{label}", bufs=4, space="PSUM") as psG, \
                     tc.tile_pool(name=f"psZ{label}", bufs=2, space="PSUM") as psZ:
                    xdT = p3.tile([F_IN, SP], f32, tag="xdT")
                    nc.sync.dma_start(out=xdT[:], in_=xdT_t[:])
                    x1T = p3.tile([128, 2, SP], f32, tag="x1T")
                    zrows = p3.tile([128, RT, CLS], f32, tag="zrows")
                    r2rows = p3.tile([128, RT, CLS], f32, tag="r2rows")
                    ngr = -(-RT // 4)
                    for g in range(ngr):
                        jj0 = 4 * g
                        njj = min(4, RT - jj0)
                        rg = njj * 128
                        aT = p3w.tile([128, 512], f32, tag="aT")
                        for q in range(njj):
                            mt = p3w.tile([128, 128], f32, tag="mt")
                            nc.vector.tensor_scalar_mul(
                                mt[:], agg_sb[:, jj0 + q, :],
                                invc_sb[:, jj0 + q:jj0 + q + 1])
                            pt = psT.tile([128, 128], f32, tag="pt")
                            nc.tensor.transpose(pt[:], mt[:], sb_ident[:])
                            nc.vector.tensor_copy(
                                aT[:, 128 * q:128 * q + 128], pt[:])
                        c0 = 512 * g
                        for h in range(2):
                            po = psG.tile([128, 512], f32, tag="po")
                            nc.tensor.matmul(
                                po[:, :rg], wl[:, 128 * h:128 * h + 128],
                                aT[:, :rg], start=True, stop=False)
                            nc.tensor.matmul(
                                po[:, :rg], wr[:, 128 * h:128 * h + 128],
                                xdT[:, c0:c0 + rg], start=False, stop=True)
                            nc.scalar.activation(
                                x1T[:, h, c0:c0 + rg], po[:, :rg], AF.Relu,
                                bias=b1k[:, h:h + 1])
                        pz = psZ.tile([128, 512], f32, tag="pz")
                        for h in range(2):
                            nc.tensor.matmul(
                                pz[0:CLS, :rg], w2l[:, h, :],
                                x1T[:, h, c0:c0 + rg],
                                start=(h == 0), stop=(h == 1))
                        for h in range(2):
                            nc.tensor.matmul(
                                pz[64:64 + CLS, :rg], w2r_o[:, h, :],
                                x1T[:, h, c0:c0 + rg],
                                start=(h == 0), stop=(h == 1))
                        zr2 = p3w.tile([128, 512], f32, tag="zr2")
                        nc.vector.tensor_copy(zr2[0:CLS, :rg], pz[0:CLS, :rg])
                        nc.vector.tensor_scalar_add(
                            zr2[64:64 + CLS, :rg], pz[64:64 + CLS, :rg],
                            b2_o[64:64 + CLS, 0:1])
                        for q in range(njj):
                            pb = psT.tile([128, 128], f32, tag="pt")
                            nc.tensor.transpose(
                                pb[:, :], zr2[:, 128 * q:128 * q + 128],
                                sb_ident[:])
                            nc.vector.tensor_copy(
                                zrows[:, jj0 + q, :], pb[:, 0:CLS])
                            nc.vector.tensor_copy(
                                r2rows[:, jj0 + q, :], pb[:, 64:64 + CLS])
                    for k0 in range(0, SP, SCAT_CHUNK):
                        nv = min(SCAT_CHUNK, S - k0)
                        if nv <= 0:
                            break
                        kt = min(SCAT_CHUNK, SP - k0) // 128
                        sl = slice(k0 // 128, k0 // 128 + kt)
                        ic = slice(k0 // 16, (k0 + kt * 128) // 16)
                        nc.gpsimd.dma_scatter_add(
                            st_z[:], zrows[:, sl, :], unp_sb[:, ic],
                            kt * 128, nv, CLS)
                        nc.gpsimd.dma_scatter_add(
                            st_r2o[:], r2rows[:, sl, :], unp_sb[:, ic],
                            kt * 128, nv, CLS)

            # ================= phase-7 emitter =================
            def phase7(agg2_sb, invc_sb, st_r2, unp_sb, t_out, label):
                with tc.tile_pool(name=f"p7{label}", bufs=1) as p7:
                    r2r = p7.tile([128, RT, CLS], f32, tag="r2r")
                    GCH = 1024
                    for k0 in range(0, SP, GCH):
                        nv = min(GCH, S - k0)
                        if nv <= 0:
                            break
                        kt = min(GCH, SP - k0) // 128
                        nc.gpsimd.dma_gather(
                            r2r[:, k0 // 128:k0 // 128 + kt, :], st_r2[:],
                            unp_sb[:, k0 // 16:(k0 + 128 * kt) // 16],
                            kt * 128, min(nv, kt * 128), CLS)
                    outt = p7.tile([128, RT, CLS], f32, tag="outt")
                    for q in range(RT):
                        tmp = p7.tile([128, CLS], f32, tag="tmp")
                        nc.vector.tensor_scalar_mul(
                            tmp[:], agg2_sb[:, q, :], invc_sb[:, q:q + 1])
                        nc.vector.tensor_tensor(
                            out=outt[:, q, :], in0=tmp[:], in1=r2r[:, q, :],
                            op=ALU.add)
                    for k0 in range(0, SP, SCAT_CHUNK):
                        nv = min(SCAT_CHUNK, S - k0)
                        if nv <= 0:
                            break
                        kt = min(SCAT_CHUNK, SP - k0) // 128
                        nc.gpsimd.dma_scatter_add(
                            t_out[:], outt[:, k0 // 128:k0 // 128 + kt, :],
                            unp_sb[:, k0 // 16:(k0 + kt * 128) // 16],
                            kt * 128, nv, CLS)

            # ================= emit the whole program =================
            import os as _os
            PARTS = set((_os.environ.get("KERNEL_PARTS") or
                         "agg1,p3,cc,agg2,p7").split(","))
            with tc.tile_pool(name="aggAp", bufs=1) as aggApool:
                aggA = aggApool.tile([128, RT, F_IN], f32)
                if "agg1" in PARTS:
                    agg_pass(sb_gidxA, TA, t_xu[CENTER:, :], F_IN, aggA, "A")
                if "p3" in PARTS:
                    phase3(aggA, t_xdTA, sb_invcA, sb_w["wu1lT"], sb_w["wu1rT"],
                           sb_w["bu1"], sb_w["wu2lT"], sb_w["wp2rT"], b2["bp2"],
                           st_zu, st_r2B, sb_unpA, "A")
            with tc.tile_pool(name="aggBp", bufs=1) as aggBpool:
                aggB = aggBpool.tile([128, RT, F_IN], f32)
                if "agg1" in PARTS:
                    agg_pass(sb_gidxB, TB, t_xp[CENTER:, :], F_IN, aggB, "B")
                if "p3" in PARTS:
                    phase3(aggB, t_xdTB, sb_invcB, sb_w["wp1lT"], sb_w["wp1rT"],
                           sb_w["bp1"], sb_w["wp2lT"], sb_w["wu2rT"], b2["bu2"],
                           st_zp, st_r2A, sb_unpB, "B")

            if "cc" not in PARTS:
                pass
            elif local_mode:
                nc.sync.dma_start(out=t_zuf[0:S, :], in_=st_zu[0:S, :])
                nc.sync.dma_start(out=t_zpf[0:S, :], in_=st_zp[0:S, :])
            else:
                nc.gpsimd.collective_compute(
                    "AllGather", mybir.AluOpType.bypass,
                    replica_groups=[list(range(NCORES))],
                    ins=[st_zu[0:S, :]], outs=[t_zuf[0:cfg.N, :]])
                nc.gpsimd.collective_compute(
                    "AllGather", mybir.AluOpType.bypass,
                    replica_groups=[list(range(NCORES))],
                    ins=[st_zp[0:S, :]], outs=[t_zpf[0:cfg.N, :]])

            with tc.tile_pool(name="agg2Ap", bufs=1) as a2p:
                agg2A = a2p.tile([128, RT, CLS], f32)
                if "agg2" in PARTS:
                    agg_pass(sb_gidxA, TA, t_zuf[CENTER:, :], CLS, agg2A, "A2")
                if "p7" in PARTS:
                    phase7(agg2A, sb_invcA, st_r2A, sb_unpA, t_xu2, "A")
            with tc.tile_pool(name="agg2Bp", bufs=1) as b2p:
                agg2B = b2p.tile([128, RT, CLS], f32)
                if "agg2" in PARTS:
                    agg_pass(sb_gidxB, TB, t_zpf[CENTER:, :], CLS, agg2B, "B2")
                if "p7" in PARTS:
                    phase7(agg2B, sb_invcB, st_r2B, sb_unpB, t_xp2, "B")

    nc.finalize()
    return nc


def build(inputs, cfg=None, local_mode=False):
    cfg = cfg or CFG()
    in_maps, TA, TB, metaA, metaB = _prep_all(inputs, cfg)
    nc = _build_nc(cfg, TA, TB, local_mode=local_mode)
    return nc, in_maps


def kernel(**inputs):
    from concourse.bass_utils import run_bass_kernel_spmd

    cfg = CFG()
    nc, in_maps = build(inputs, cfg)
    res = run_bass_kernel_spmd(nc, in_maps, list(range(NCORES)))
    xu2 = np.concatenate(
        [res.results[c]["xu2"][: cfg.S] for c in range(NCORES)], 0)
    xp2 = np.concatenate(
        [res.results[c]["xp2"][: cfg.S] for c in range(NCORES)], 0)
    return xu2, xp2



# revision 28
# speedup vs baseline: 1.7255x; 1.7255x over previous
"""Bipartite 2-layer SAGEConv GNN on 8 Trainium2 NeuronCores.

Strategy (v2):
  - Edges sharded by destination range (core c owns dst rows [S*c, S*(c+1))
    for BOTH directions, so layer-2 lin_r terms stay core-local).
  - Per core+direction, dsts sorted by degree; edges packed into SEG-slot
    segments, BPD dst-rows per tile (SEG*BPD=128), variable tiles per block
    (schedule = max over cores, so one SPMD program serves all cores).
  - Message tables in bf16; gather with CENTERED base (int16 idx).
  - Segment-MEAN via PE matmul: lhsT = message tile [128 slots, F],
    rhs = per-block one-hot scaled by 1/deg (bf16) -> aggT [F, BPD] in PSUM.
    Feature-major output means NO transposes before the dense GEMMs.
  - Dense layer GEMMs in bf16 with f32 PSUM accumulation; relu on Act.
  - Layer 2 transform-first: z_cat[i] = [x?_1[i] @ w2l.T]: dir A fills
    cols 0:64 (zu), dir B cols 64:128 (zp) of one bf16 table -> one
    AllGather, 256B gather rows.
  - All permutations composed into gather indices; outputs stored permuted
    and un-permuted on the HOST (no scatter-adds anywhere).
"""
import sys
import numpy as np
import ml_dtypes

sys.path.insert(0, "/opt/trn_rl_repo")

BF16 = ml_dtypes.bfloat16

# ---------------- problem dims (hardcoded for the harness) ----------------
N = 50000
E = 800000
F_IN = 128
HID = 256
CLS = 64
NCORES = 8

SEG = 2            # slots per segment row per tile
BPD = 128 // SEG   # dsts per tile/block
GB = 512 // BPD    # blocks per phase3/7 group (512 dst rows)
CHUNK_TILES = 8    # tiles per gather call (1024 idx = HW SWDGE ring limit)
SCRATCH = 49152    # SWDGE ring carveout bytes/partition (48KB HW budget)


def set_chunk(tiles, scratch):
    global CHUNK_TILES, SCRATCH
    CHUNK_TILES, SCRATCH = tiles, scratch


class CFG:
    def __init__(self, n=N, center=None):
        self.N = n
        self.S = n // NCORES          # dst rows per core
        self.CENTER = n // 2 if center is None else center
        self.ZROW = n                 # zero row index in the tables
        self.NB = -(-self.S // BPD)   # blocks per direction
        self.RT = -(-self.S // 128)   # 128-row tiles of the slice
        self.SP = self.RT * 128       # padded rows
        self.NG = -(-self.NB // GB)   # phase3/7 groups


# ---------------- host-side edge scheduling ----------------

def _prep_dir(src_g, dst_g, c, cfg):
    lo = c * cfg.S
    m = (dst_g >= lo) & (dst_g < lo + cfg.S)
    ls = src_g[m].astype(np.int64)
    ld = (dst_g[m] - lo).astype(np.int64)
    deg = np.bincount(ld, minlength=cfg.S)
    pi = np.argsort(-deg, kind="stable").astype(np.int64)
    order = np.argsort(ld, kind="stable")
    ls_s = ls[order]
    starts = np.zeros(cfg.S + 1, np.int64)
    starts[1:] = np.cumsum(deg)
    degp = np.zeros(cfg.NB * BPD, np.int64)
    degp[: cfg.S] = deg[pi]
    treq = np.maximum(
        1, -(-degp.reshape(cfg.NB, BPD).max(1) // SEG)
    ).astype(np.int64)
    return dict(pi=pi, deg=deg, starts=starts, ls_s=ls_s, treq=treq)


def _tail_tiles(total_tiles):
    """Tile indices whose partition-127 slot must map >= CENTER (the SWDGE
    ucode trims trailing-negative indices per gather call)."""
    ts = set(range(CHUNK_TILES - 1, total_tiles, CHUNK_TILES))
    ts.add(total_tiles - 1)
    return sorted(ts)


def _build_slots(meta, T, cfg, allow_pi_swap):
    """Slot array [ntiles, 128] of raw node ids (ZROW for dummies).

    Fixes gather-call tails so the final slot of every call is >= CENTER,
    preferring within-row (same dst) swaps; falls back to swapping dst rows
    within the block (mutating meta["pi"]) when allowed.
    """
    pi, deg, starts, ls_s = meta["pi"], meta["deg"], meta["starts"], meta["ls_s"]
    total_tiles = int(T.sum())
    out = np.full((total_tiles, BPD, SEG), cfg.ZROW, np.int64)
    row_of_tile = np.zeros(total_tiles, np.int64)
    t0 = 0
    blk_start = {}
    for b in range(cfg.NB):
        tb = int(T[b])
        blk_start[b] = t0
        row_of_tile[t0:t0 + tb] = b
        blk = out[t0:t0 + tb]
        for mrow in range(BPD):
            r = BPD * b + mrow
            if r >= cfg.S:
                continue
            D = int(pi[r])
            d = int(deg[D])
            if d == 0:
                continue
            vals = np.full(tb * SEG, cfg.ZROW, np.int64)
            vals[:d] = ls_s[starts[D]: starts[D] + d]
            blk[:, mrow, :] = vals.reshape(tb, SEG)
        t0 += tb

    tails_in_block = {}
    for tg in _tail_tiles(total_tiles):
        b = int(row_of_tile[tg])
        tails_in_block.setdefault(b, []).append(tg - blk_start[b])
    for tg in _tail_tiles(total_tiles):
        b = int(row_of_tile[tg])
        tb = int(T[b])
        blk = out[blk_start[b]: blk_start[b] + tb]
        tl = tg - blk_start[b]
        jlast = tl * SEG + SEG - 1
        # positions guarding ANY tail in this block's last row are off-limits
        # as swap sources (a prior/later tail fix depends on them)
        jtails = {t * SEG + SEG - 1 for t in tails_in_block[b]}
        if blk[tl, BPD - 1, SEG - 1] >= cfg.CENTER:
            continue
        flat = blk[:, BPD - 1, :].reshape(-1).copy()
        good = [int(j) for j in np.nonzero(flat >= cfg.CENTER)[0]
                if int(j) not in jtails]
        if not good:
            assert allow_pi_swap, "L2 tail fix needs a pi swap; not supported"
            mgood = -1
            for mm in range(BPD - 2, -1, -1):
                if (blk[:, mm, :] >= cfg.CENTER).any():
                    mgood = mm
                    break
            assert mgood >= 0, "no non-negative slot available for chunk tail"
            r1, r2 = BPD * b + mgood, BPD * b + BPD - 1
            pi[r1], pi[r2] = pi[r2], pi[r1]
            tmpv = blk[:, mgood, :].copy()
            blk[:, mgood, :] = blk[:, BPD - 1, :]
            blk[:, BPD - 1, :] = tmpv
            flat = blk[:, BPD - 1, :].reshape(-1).copy()
            good = [int(j) for j in np.nonzero(flat >= cfg.CENTER)[0]
                    if int(j) not in jtails]
            assert good, "no swappable non-negative slot for chunk tail"
        j = good[-1]
        flat[j], flat[jlast] = flat[jlast], flat[j]
        blk[:, BPD - 1, :] = flat.reshape(tb, SEG)
    return out.reshape(total_tiles, 128)


def _fix_tails_inplace(slots, T, cfg):
    """Within-row-only tail fix for an already-composed slot array."""
    total_tiles = slots.shape[0]
    sl = slots.reshape(total_tiles, BPD, SEG)
    row_of_tile = np.repeat(np.arange(cfg.NB), T)
    starts = np.zeros(cfg.NB, np.int64)
    starts[1:] = np.cumsum(T)[:-1]
    tails_in_block = {}
    for tg in _tail_tiles(total_tiles):
        b = int(row_of_tile[tg])
        tails_in_block.setdefault(b, []).append(tg - starts[b])
    for tg in _tail_tiles(total_tiles):
        b = int(row_of_tile[tg])
        tb = int(T[b])
        blk = sl[starts[b]: starts[b] + tb]
        tl = tg - starts[b]
        jlast = tl * SEG + SEG - 1
        jtails = {t * SEG + SEG - 1 for t in tails_in_block[b]}
        if blk[tl, BPD - 1, SEG - 1] >= cfg.CENTER:
            continue
        flat = blk[:, BPD - 1, :].reshape(-1).copy()
        good = [int(j) for j in np.nonzero(flat >= cfg.CENTER)[0]
                if int(j) not in jtails]
        assert good, "L2 tail fix impossible (within-row)"
        j = good[-1]
        flat[j], flat[jlast] = flat[jlast], flat[j]
        blk[:, BPD - 1, :] = flat.reshape(tb, SEG)


def _wrap16(idx16):
    n = len(idx16)
    assert n % 16 == 0
    return np.tile(idx16.reshape(n // 16, 16).T, (8, 1)).astype(np.int16)


def _gidx(slots, cfg):
    nt = slots.shape[0]
    ntp = -(-nt // CHUNK_TILES) * CHUNK_TILES
    slp = np.full((ntp, 128), cfg.ZROW, np.int64)
    slp[:nt] = slots
    return _wrap16((slp.reshape(-1) - cfg.CENTER).astype(np.int16))


def _r8s(meta, cfg):
    """[128, NB*BPD] bf16: one-hot scaled by 1/deg (segment-mean weights)."""
    degp = np.zeros(cfg.NB * BPD, np.int64)
    degp[: cfg.S] = meta["deg"][meta["pi"]]
    invc = np.zeros(cfg.NB * BPD, np.float32)
    invc[: cfg.S] = 1.0 / np.maximum(degp[: cfg.S], 1.0)
    r = np.zeros((128, cfg.NB * BPD), np.float32)
    cols = np.arange(cfg.NB * BPD)
    d = cols % BPD
    for s in range(SEG):
        r[d * SEG + s, cols] = invc
    return r.astype(BF16)


def _prep_all(inputs, cfg):
    x_user = np.asarray(inputs["x_user"], np.float32)
    x_product = np.asarray(inputs["x_product"], np.float32)
    ei = np.asarray(inputs["edge_index"]).astype(np.int64)
    u, p = ei[0], ei[1]

    metaA = [_prep_dir(u, p, c, cfg) for c in range(NCORES)]  # dst=p, src=u
    metaB = [_prep_dir(p, u, c, cfg) for c in range(NCORES)]  # dst=u, src=p

    TA = np.max([m["treq"] for m in metaA], axis=0)
    TB = np.max([m["treq"] for m in metaB], axis=0)

    # L1 slot arrays first (may mutate pi); then composed maps from final pi
    slotsA = [_build_slots(metaA[c], TA, cfg, True) for c in range(NCORES)]
    slotsB = [_build_slots(metaB[c], TB, cfg, True) for c in range(NCORES)]

    # invmap[node] = owner*S + pi_inv[owner][node_local] (per direction)
    def make_invmap(metas):
        inv = np.zeros(cfg.N + 1, np.int64)
        for c in range(NCORES):
            pi = metas[c]["pi"]
            pinv = np.zeros(cfg.S, np.int64)
            pinv[pi] = np.arange(cfg.S)
            inv[c * cfg.S:(c + 1) * cfg.S] = c * cfg.S + pinv
        inv[cfg.N] = cfg.N
        return inv

    invA = make_invmap(metaA)   # where node's zu row lives
    invB = make_invmap(metaB)   # where node's zp row lives

    def tab(x):
        t = np.zeros((cfg.N + 1, F_IN), BF16)
        t[: cfg.N] = x.astype(BF16)
        return t

    w = {k: np.asarray(v, np.float32) for k, v in inputs.items()
         if k.startswith(("w_", "b_"))}
    shared = {
        "xu_tab": tab(x_user), "xp_tab": tab(x_product),
        "wu1lT": np.ascontiguousarray(w["w_u1_l"].T).astype(BF16),
        "wu1rT": np.ascontiguousarray(w["w_u1_r"].T).astype(BF16),
        "wp1lT": np.ascontiguousarray(w["w_p1_l"].T).astype(BF16),
        "wp1rT": np.ascontiguousarray(w["w_p1_r"].T).astype(BF16),
        "wu2lT": np.ascontiguousarray(w["w_u2_l"].T).astype(BF16),
        "wu2rT": np.ascontiguousarray(w["w_u2_r"].T).astype(BF16),
        "wp2lT": np.ascontiguousarray(w["w_p2_l"].T).astype(BF16),
        "wp2rT": np.ascontiguousarray(w["w_p2_r"].T).astype(BF16),
        "bu1": np.ascontiguousarray(w["b_u1"].reshape(2, 128).T),
        "bp1": np.ascontiguousarray(w["b_p1"].reshape(2, 128).T),
        "bu2": np.ascontiguousarray(w["b_u2"].reshape(CLS, 1)),
        "bp2": np.ascontiguousarray(w["b_p2"].reshape(CLS, 1)),
        "ident": np.eye(128, dtype=np.float32),
    }

    in_maps = []
    metas = []
    for c in range(NCORES):
        d = dict(shared)
        mA, mB = metaA[c], metaB[c]
        d["gidx1A"] = _gidx(slotsA[c], cfg)
        d["gidx1B"] = _gidx(slotsB[c], cfg)
        # L2: composed indices into the z table (invA for dir A msgs = zu rows)
        s2A = invA[slotsA[c]]
        s2B = invB[slotsB[c]]
        _fix_tails_inplace(s2A, TA, cfg)
        _fix_tails_inplace(s2B, TB, cfg)
        d["gidx2A"] = _gidx(s2A, cfg)
        d["gidx2B"] = _gidx(s2B, cfg)
        # phase7 r2 gather: r2A stored piB-ordered, needed piA-ordered (and
        # vice versa): idx[r] = piOther_inv[piX[r]]
        piAinv = np.zeros(cfg.S, np.int64); piAinv[mA["pi"]] = np.arange(cfg.S)
        piBinv = np.zeros(cfg.S, np.int64); piBinv[mB["pi"]] = np.arange(cfg.S)
        gr2A = np.zeros(cfg.SP, np.int64)
        gr2A[: cfg.S] = piBinv[mA["pi"]]
        gr2B = np.zeros(cfg.SP, np.int64)
        gr2B[: cfg.S] = piAinv[mB["pi"]]
        d["gr2A"] = _wrap16(gr2A.astype(np.int16))
        d["gr2B"] = _wrap16(gr2B.astype(np.int16))
        d["r8sA"] = _r8s(mA, cfg)
        d["r8sB"] = _r8s(mB, cfg)
        for tag, meta, xsrc in (("A", mA, x_product), ("B", mB, x_user)):
            rows = c * cfg.S + meta["pi"]
            xdT = np.zeros((F_IN, cfg.SP), BF16)
            xdT[:, : cfg.S] = xsrc[rows].T.astype(BF16)
            d[f"xdT{tag}"] = xdT
        in_maps.append(d)
        metas.append(dict(piA=mA["pi"].copy(), piB=mB["pi"].copy()))

    return in_maps, TA, TB, metas


# ---------------- device program ----------------

def _build_nc(cfg, TA, TB, local_mode=False):
    import concourse.bacc as bacc
    import concourse.mybir as mybir
    from concourse.tile import TileContext

    f32, i16, bf16 = mybir.dt.float32, mybir.dt.int16, mybir.dt.bfloat16
    AF = mybir.ActivationFunctionType
    ALU = mybir.AluOpType

    nc = bacc.Bacc(None, target_bir_lowering=False, num_devices=NCORES,
                   dynamic_dma_scratch_size=SCRATCH, num_swdge_queues=1)

    S, SP, RT, NB, NG, CENTER = cfg.S, cfg.SP, cfg.RT, cfg.NB, cfg.NG, cfg.CENTER
    ntA, ntB = int(TA.sum()), int(TB.sum())

    def colsn(nt):
        return -(-nt // CHUNK_TILES) * CHUNK_TILES * 8

    # ---- DRAM declarations ----
    t_xu = nc.dram_tensor("xu_tab", [cfg.N + 1, F_IN], bf16, kind="ExternalInput")
    t_xp = nc.dram_tensor("xp_tab", [cfg.N + 1, F_IN], bf16, kind="ExternalInput")
    tw = {}
    for k in ["wu1lT", "wu1rT", "wp1lT", "wp1rT"]:
        tw[k] = nc.dram_tensor(k, [F_IN, HID], bf16, kind="ExternalInput")
    for k in ["wu2lT", "wu2rT", "wp2lT", "wp2rT"]:
        tw[k] = nc.dram_tensor(k, [HID, CLS], bf16, kind="ExternalInput")
    for k in ["bu1", "bp1"]:
        tw[k] = nc.dram_tensor(k, [128, 2], f32, kind="ExternalInput")
    for k in ["bu2", "bp2"]:
        tw[k] = nc.dram_tensor(k, [CLS, 1], f32, kind="ExternalInput")
    t_ident = nc.dram_tensor("ident", [128, 128], f32, kind="ExternalInput")
    t_g1A = nc.dram_tensor("gidx1A", [128, colsn(ntA)], i16, kind="ExternalInput")
    t_g1B = nc.dram_tensor("gidx1B", [128, colsn(ntB)], i16, kind="ExternalInput")
    t_g2A = nc.dram_tensor("gidx2A", [128, colsn(ntA)], i16, kind="ExternalInput")
    t_g2B = nc.dram_tensor("gidx2B", [128, colsn(ntB)], i16, kind="ExternalInput")
    t_gr2A = nc.dram_tensor("gr2A", [128, SP // 16], i16, kind="ExternalInput")
    t_gr2B = nc.dram_tensor("gr2B", [128, SP // 16], i16, kind="ExternalInput")
    t_r8sA = nc.dram_tensor("r8sA", [128, NB * BPD], bf16, kind="ExternalInput")
    t_r8sB = nc.dram_tensor("r8sB", [128, NB * BPD], bf16, kind="ExternalInput")
    t_xdTA = nc.dram_tensor("xdTA", [F_IN, SP], bf16, kind="ExternalInput")
    t_xdTB = nc.dram_tensor("xdTB", [F_IN, SP], bf16, kind="ExternalInput")

    # outputs partition-major: row (128*q + p) of the permuted slice lives at
    # [p, q, :] -> per-partition contiguous runs (1KB descriptors) on store
    t_xu2 = nc.dram_tensor("xu2", [128, RT, CLS], f32, kind="ExternalOutput")
    t_xp2 = nc.dram_tensor("xp2", [128, RT, CLS], f32, kind="ExternalOutput")

    st_z = nc.dram_tensor("z_stage", [SP, 2 * CLS], bf16)
    st_r2A = nc.dram_tensor("r2A_stage", [SP, CLS], f32)
    st_r2B = nc.dram_tensor("r2B_stage", [SP, CLS], f32)
    aspace = "Local" if local_mode else "Shared"
    t_zf = nc.dram_tensor("z_full", [cfg.N + 1, 2 * CLS], bf16, addr_space=aspace)

    import os as _os
    DBG = bool(_os.environ.get("KERNEL_DEBUG"))
    t_dbg = {}
    if DBG:
        t_dbg["aggT_A"] = nc.dram_tensor("dbg_aggT_A", [128, NG, 512], bf16,
                                         kind="ExternalOutput")
        t_dbg["x1_A"] = nc.dram_tensor("dbg_x1_A", [128, NG, 2, 512], bf16,
                                       kind="ExternalOutput")
        t_dbg["zst"] = nc.dram_tensor("dbg_zst", [SP, 2 * CLS], bf16,
                                      kind="ExternalOutput")
        t_dbg["r2A"] = nc.dram_tensor("dbg_r2A", [SP, CLS], f32,
                                      kind="ExternalOutput")
        t_dbg["r2B"] = nc.dram_tensor("dbg_r2B", [SP, CLS], f32,
                                      kind="ExternalOutput")
        t_dbg["aggT2_A"] = nc.dram_tensor("dbg_aggT2_A", [128, NG, 512], f32,
                                          kind="ExternalOutput")
        t_dbg["r2r_A"] = nc.dram_tensor("dbg_r2r_A", [128, RT, CLS], f32,
                                        kind="ExternalOutput")

    ST_Z = st_z.rearrange("(c p) f -> p c f", p=128)
    ST_R2 = {"A": st_r2A.rearrange("(c p) f -> p c f", p=128),
             "B": st_r2B.rearrange("(c p) f -> p c f", p=128)}
    OUT = {"A": t_xu2, "B": t_xp2}

    with TileContext(nc) as tc:
        with tc.tile_pool(name="persist", bufs=1) as pp:
            sb_ident = pp.tile([128, 128], f32)
            sb_r8s = {"A": pp.tile([128, NB * BPD], bf16, name="r8sA"),
                      "B": pp.tile([128, NB * BPD], bf16, name="r8sB")}
            sb_w = {}
            for k in ["wu1lT", "wu1rT", "wp1lT", "wp1rT"]:
                sb_w[k] = pp.tile([F_IN, HID], bf16, tag=k, name=k)
            for k in ["wu2lT", "wu2rT", "wp2lT", "wp2rT"]:
                sb_w[k] = pp.tile([128, 2, CLS], bf16, tag=k, name=k)
            for k in ["bu1", "bp1"]:
                sb_w[k] = pp.tile([128, 2], f32, tag=k, name=k)
            for k in ["bu2", "bp2"]:
                # loaded at partitions [64:128] to align with the r2 half
                sb_w[k] = pp.tile([128, 1], f32, tag=k, name=k)

            nc.sync.dma_start(out=sb_ident[:], in_=t_ident[:])
            nc.sync.dma_start(out=sb_r8s["A"][:], in_=t_r8sA[:])
            nc.sync.dma_start(out=sb_r8s["B"][:], in_=t_r8sB[:])
            for k, t in tw.items():
                if k in ("wu2lT", "wu2rT", "wp2lT", "wp2rT"):
                    nc.sync.dma_start(
                        out=sb_w[k][:],
                        in_=t.rearrange("(k p) c -> p k c", p=128)[:])
                elif k in ("bu2", "bp2"):
                    nc.sync.dma_start(out=sb_w[k][64:64 + CLS, :], in_=t[:])
                else:
                    nc.sync.dma_start(out=sb_w[k][:], in_=t[:])

            # zero row of the z table
            with tc.tile_pool(name="zrow0", bufs=1) as zp0:
                zt = zp0.tile([128, 2 * CLS], bf16)
                nc.vector.memset(zt[:], 0.0)
                nc.sync.dma_start(out=t_zf[cfg.N:cfg.N + 1, :], in_=zt[0:1, :])

            # ============ layer-1 phase emitter (agg + dense, pipelined) ====
            def l1_phase(tag, T, gidx_t, table_ap, xdT_t, wl, wr, b1, w2l,
                         w2r_o, b2_o, st_r2o_tag, zhalf, pools):
                (gp, xp_, mp, aggp, x1p, zrp, zwp, rwp,
                 psA, psG, psZ, psT) = pools
                ntiles = int(T.sum())
                sb_gidx = gp.tile([128, colsn(ntiles)], i16, tag="gidx",
                                  name=f"gidx{tag}")
                nc.sync.dma_start(out=sb_gidx[:], in_=gidx_t[:])
                xdT = xp_.tile([128, SP], bf16, tag="xdT", name=f"xdT{tag}")
                nc.sync.dma_start(out=xdT[:], in_=xdT_t[:])
                r8 = sb_r8s[tag]
                msgs = {}

                def chunk_of(tg):
                    ch = tg // CHUNK_TILES
                    if ch not in msgs:
                        t0c = ch * CHUNK_TILES
                        ct = min(CHUNK_TILES, ntiles - t0c)
                        m = mp.tile([128, CHUNK_TILES, F_IN], bf16,
                                    tag="msg", name=f"msg{tag}_{ch}")
                        nc.gpsimd.dma_gather(
                            m[:, :ct, :], table_ap,
                            sb_gidx[:, 8 * t0c:8 * t0c + 8 * ct],
                            ct * 128, ct * 128, F_IN)
                        msgs[ch] = m
                    return msgs[ch]

                tg = 0
                for g in range(NG):
                    b0 = g * GB
                    nb = min(GB, NB - b0)
                    rg = BPD * nb
                    aggT = aggp.tile([128, 512], bf16, tag="aggT")
                    for j in range(nb):
                        b = b0 + j
                        # one PSUM bank per accumulation group: start_tensor_calc
                        # pending-zeroes the whole 2KB zero region
                        ps = psA.tile([128, BPD], f32, tag="ps")
                        for k in range(int(T[b])):
                            m = chunk_of(tg)
                            nc.tensor.matmul(
                                ps[:],
                                m[:, tg % CHUNK_TILES, :],
                                r8[:, BPD * b:BPD * b + BPD],
                                start=(k == 0), stop=(k == int(T[b]) - 1))
                            tg += 1
                        nc.vector.tensor_copy(
                            aggT[:, BPD * j:BPD * (j + 1)], ps[:])
                    # dense: x1 = relu(wl.T @ aggT + wr.T @ xdT + b1)
                    cg = 512 * g
                    x1 = x1p.tile([128, 2, 512], bf16, tag="x1")
                    for h in range(2):
                        po = psG.tile([128, 512], f32, tag="po")
                        nc.tensor.matmul(po[:, :rg],
                                         wl[:, 128 * h:128 * h + 128],
                                         aggT[:, :rg], start=True, stop=False)
                        nc.tensor.matmul(po[:, :rg],
                                         wr[:, 128 * h:128 * h + 128],
                                         xdT[:, cg:cg + rg],
                                         start=False, stop=True)
                        nc.scalar.activation(x1[:, h, :rg], po[:, :rg],
                                             AF.Relu, bias=b1[:, h:h + 1])
                    pz = psZ.tile([128, 512], f32, tag="pz")
                    for h in range(2):
                        nc.tensor.matmul(pz[0:CLS, :rg], w2l[:, h, :],
                                         x1[:, h, :rg],
                                         start=(h == 0), stop=(h == 1))
                    for h in range(2):
                        nc.tensor.matmul(pz[CLS:2 * CLS, :rg], w2r_o[:, h, :],
                                         x1[:, h, :rg],
                                         start=(h == 0), stop=(h == 1))
                    zr2 = zrp.tile([128, 512], f32, tag="zr2")
                    nc.vector.tensor_copy(zr2[0:CLS, :rg], pz[0:CLS, :rg])
                    # fold the other direction's layer-2 bias into its r2 rows
                    nc.vector.tensor_scalar_add(
                        zr2[CLS:2 * CLS, :rg], pz[CLS:2 * CLS, :rg],
                        b2_o[CLS:2 * CLS, 0:1])
                    nq = rg // 128
                    zrow = zwp.tile([128, 4, CLS], bf16, tag="zrow")
                    r2row = rwp.tile([128, 4, CLS], f32, tag="r2row")
                    for q in range(nq):
                        pt = psT.tile([128, 128], f32, tag="pt")
                        nc.tensor.transpose(pt[:], zr2[:, 128 * q:128 * q + 128],
                                            sb_ident[:])
                        nc.vector.tensor_copy(zrow[:, q, :], pt[:, 0:CLS])
                        nc.vector.tensor_copy(r2row[:, q, :], pt[:, CLS:2 * CLS])
                    q0 = 4 * g
                    nc.sync.dma_start(
                        out=ST_Z[:, q0:q0 + nq, CLS * zhalf:CLS * zhalf + CLS],
                        in_=zrow[:, :nq, :])
                    nc.sync.dma_start(
                        out=ST_R2[st_r2o_tag][:, q0:q0 + nq, :],
                        in_=r2row[:, :nq, :])
                    if DBG and tag == "A":
                        nc.sync.dma_start(out=t_dbg["aggT_A"][:, g, :],
                                          in_=aggT[:])
                        nc.sync.dma_start(out=t_dbg["x1_A"][:, g, :, :],
                                          in_=x1[:])

            # ============ layer-2 phase emitter ============
            def l2_phase(tag, T, gidx_t, gr2_t, st_r2_t, zhalf, t_out,
                         pools):
                gp, mp, r2p, agp, otp, psA, psT = pools
                ntiles = int(T.sum())
                sb_gidx = gp.tile([128, colsn(ntiles)], i16, tag="gidx",
                                  name=f"gidx2{tag}")
                nc.sync.dma_start(out=sb_gidx[:], in_=gidx_t[:])
                sb_gr2 = gp.tile([128, SP // 16], i16, tag="gr2",
                                 name=f"gr2{tag}")
                nc.sync.dma_start(out=sb_gr2[:], in_=gr2_t[:])
                r2r = r2p.tile([128, RT, CLS], f32, tag="r2r",
                               name=f"r2r{tag}")
                GCH = CHUNK_TILES * 128
                for k0 in range(0, SP, GCH):
                    kt = min(GCH, SP - k0) // 128
                    nc.gpsimd.dma_gather(
                        r2r[:, k0 // 128:k0 // 128 + kt, :], st_r2_t[:],
                        sb_gr2[:, k0 // 16:(k0 + 128 * kt) // 16],
                        kt * 128, kt * 128, CLS)
                r8 = sb_r8s[tag]
                msgs = {}

                def chunk_of(tg):
                    ch = tg // CHUNK_TILES
                    if ch not in msgs:
                        t0c = ch * CHUNK_TILES
                        ct = min(CHUNK_TILES, ntiles - t0c)
                        m = mp.tile([128, CHUNK_TILES, 2 * CLS], bf16,
                                    tag="msg", name=f"msg2{tag}_{ch}")
                        nc.gpsimd.dma_gather(
                            m[:, :ct, :], t_zf[CENTER:, :],
                            sb_gidx[:, 8 * t0c:8 * t0c + 8 * ct],
                            ct * 128, ct * 128, 2 * CLS)
                        msgs[ch] = m
                    return msgs[ch]

                tg = 0
                for g in range(NG):
                    b0 = g * GB
                    nb = min(GB, NB - b0)
                    rg = BPD * nb
                    aggT2 = agp.tile([128, 512], f32, tag="aggT2")
                    for j in range(nb):
                        b = b0 + j
                        ps = psA.tile([128, BPD], f32, tag="ps2")
                        for k in range(int(T[b])):
                            m = chunk_of(tg)
                            nc.tensor.matmul(
                                ps[:],
                                m[:, tg % CHUNK_TILES, :],
                                r8[:, BPD * b:BPD * b + BPD],
                                start=(k == 0), stop=(k == int(T[b]) - 1))
                            tg += 1
                        nc.vector.tensor_copy(
                            aggT2[:, BPD * j:BPD * (j + 1)], ps[:])
                    nq = rg // 128
                    outt = otp.tile([128, 4, CLS], f32, tag="outt")
                    for q in range(nq):
                        pt = psT.tile([128, 128], f32, tag="pt2")
                        nc.tensor.transpose(pt[:],
                                            aggT2[:, 128 * q:128 * q + 128],
                                            sb_ident[:])
                        nc.vector.tensor_tensor(
                            out=outt[:, q, :],
                            in0=pt[:, CLS * zhalf:CLS * zhalf + CLS],
                            in1=r2r[:, 4 * g + q, :], op=ALU.add)
                    q0 = 4 * g
                    nc.sync.dma_start(out=OUT[tag][:, q0:q0 + nq, :],
                                      in_=outt[:, :nq, :])
                    if DBG and tag == "A":
                        nc.sync.dma_start(out=t_dbg["aggT2_A"][:, g, :],
                                          in_=aggT2[:])
                        if g == 0:
                            nc.sync.dma_start(out=t_dbg["r2r_A"][:], in_=r2r[:])

            # ================= emit =================
            with tc.tile_pool(name="l1g", bufs=2) as gp, \
                 tc.tile_pool(name="l1x", bufs=2) as xp_, \
                 tc.tile_pool(name="l1m", bufs=4) as mp, \
                 tc.tile_pool(name="l1agg", bufs=3) as aggp, \
                 tc.tile_pool(name="l1x1", bufs=2) as x1p, \
                 tc.tile_pool(name="l1zr", bufs=2) as zrp, \
                 tc.tile_pool(name="l1zw", bufs=2) as zwp, \
                 tc.tile_pool(name="l1rw", bufs=2) as rwp, \
                 tc.tile_pool(name="psA", bufs=2, space="PSUM") as psA, \
                 tc.tile_pool(name="psG", bufs=2, space="PSUM") as psG, \
                 tc.tile_pool(name="psZ", bufs=2, space="PSUM") as psZ, \
                 tc.tile_pool(name="psT", bufs=2, space="PSUM") as psT:
                pools = (gp, xp_, mp, aggp, x1p, zrp, zwp, rwp,
                         psA, psG, psZ, psT)
                l1_phase("A", TA, t_g1A, t_xu[CENTER:, :], t_xdTA,
                         sb_w["wu1lT"], sb_w["wu1rT"], sb_w["bu1"],
                         sb_w["wu2lT"], sb_w["wp2rT"], sb_w["bp2"],
                         "B", 0, pools)
                l1_phase("B", TB, t_g1B, t_xp[CENTER:, :], t_xdTB,
                         sb_w["wp1lT"], sb_w["wp1rT"], sb_w["bp1"],
                         sb_w["wp2lT"], sb_w["wu2rT"], sb_w["bu2"],
                         "A", 1, pools)

            if DBG:
                nc.sync.dma_start(out=t_dbg["zst"][:], in_=st_z[:])
                nc.sync.dma_start(out=t_dbg["r2A"][:], in_=st_r2A[:])
                nc.sync.dma_start(out=t_dbg["r2B"][:], in_=st_r2B[:])
            if local_mode:
                nc.sync.dma_start(out=t_zf[0:S, :], in_=st_z[0:S, :])
            else:
                import concourse.mybir as _mb
                nc.gpsimd.collective_compute(
                    "AllGather", _mb.AluOpType.bypass,
                    replica_groups=[list(range(NCORES))],
                    ins=[st_z[0:S, :]], outs=[t_zf[0:cfg.N, :]])

            with tc.tile_pool(name="l2g", bufs=2) as gp2, \
                 tc.tile_pool(name="l2m", bufs=4) as mp2, \
                 tc.tile_pool(name="l2r2", bufs=2) as r2p, \
                 tc.tile_pool(name="l2agg", bufs=3) as agp, \
                 tc.tile_pool(name="l2out", bufs=2) as otp, \
                 tc.tile_pool(name="psA2", bufs=3, space="PSUM") as psA2, \
                 tc.tile_pool(name="psT2", bufs=2, space="PSUM") as psT2:
                pools2 = (gp2, mp2, r2p, agp, otp, psA2, psT2)
                l2_phase("A", TA, t_g2A, t_gr2A, st_r2A, 0, t_xu2, pools2)
                l2_phase("B", TB, t_g2B, t_gr2B, st_r2B, 1, t_xp2, pools2)

    nc.finalize()
    return nc


def build(inputs, cfg=None, local_mode=False):
    cfg = cfg or CFG()
    in_maps, TA, TB, metas = _prep_all(inputs, cfg)
    nc = _build_nc(cfg, TA, TB, local_mode=local_mode)
    return nc, in_maps, metas


def postprocess(results, metas, cfg=None):
    """results: per-core dicts with permuted xu2/xp2; undo pi on the host."""
    cfg = cfg or CFG()
    xu2 = np.zeros((N, CLS), np.float32)
    xp2 = np.zeros((N, CLS), np.float32)
    for c in range(NCORES):
        piA, piB = metas[c]["piA"], metas[c]["piB"]
        # device layout [128, RT, CLS]: permuted row 128*q + p at [p, q, :]
        ru = np.asarray(results[c]["xu2"]).transpose(1, 0, 2).reshape(cfg.SP, CLS)
        rp = np.asarray(results[c]["xp2"]).transpose(1, 0, 2).reshape(cfg.SP, CLS)
        xu2[c * cfg.S + piA] = ru[: cfg.S]
        xp2[c * cfg.S + piB] = rp[: cfg.S]
    return xu2, xp2


def kernel(**inputs):
    from concourse.bass_utils import run_bass_kernel_spmd

    cfg = CFG()
    nc, in_maps, metas = build(inputs, cfg)
    res = run_bass_kernel_spmd(nc, in_maps, list(range(NCORES)))
    return postprocess(res.results, metas, cfg)
